# revision 38
# baseline (speedup 1.0000x reference)
"""MLA absorbed-QKVO attention kernel for Trainium2 (8 NeuronCores), v3.

Sharding: heads (H=16) tensor-parallel across 8 cores, 2 heads/core.
Host fuses W_h = w_qb_h @ w_qa (per-core), pre-transposes/casts all
weights + hidden to bf16, and builds swizzled rope tables + causal
masks. Each core computes a partial output (its 2 heads through w_o);
the host sums the 8 partials.

Device dataflow is weights-stationary / d-major throughout:
  q^T       = W_hT  x hidT      (PSUM -> queryT slots, rope on d-rows)
  kv^T      = w_kvT x hidT      (PSUM -> keyT slots + V via PE transpose)
  scores^T  = keyT^T x queryT   (PSUM -> exp -> P^T bf16, no-max softmax)
  attnout   = P^T^T x V         (q-major PSUM; 1/sum via per-partition
                                 scale at evacuation; ones-column of V
                                 gives the softmax denominator for free)
  out       = attnoutT^T x w_oT (after a small PE transpose of attnout)
"""

import sys

import numpy as np

if "/opt/trn_rl_repo" not in sys.path:
    sys.path.insert(0, "/opt/trn_rl_repo")

import ml_dtypes

BF = ml_dtypes.bfloat16

B, S, HID = 2, 2048, 2048
H = 16
QK_ROPE = 64
KVR = 512
QLR = 1536
KVD = 640
DHEAD = 576
N_CORES = 8
HPC = H // N_CORES
OC = HPC * DHEAD      # 1152
OCP = HPC * 640       # 1280 (per-head padded to 5x128)
SCALE = 1.0 / float(np.sqrt(128.0))

P = 128
SBLK = 512


def build_nc(b_count=B, s_len=S, debug=False, stage=3):
    import concourse.bass as bass  # noqa: F401
    import concourse.mybir as mybir
    import concourse.tile as tile
    from concourse import bacc
    from concourse.masks import make_identity

    fp32 = mybir.dt.float32
    bf16 = mybir.dt.bfloat16
    Exp = mybir.ActivationFunctionType.Exp
    Copy = mybir.ActivationFunctionType.Copy

    NB = s_len // SBLK          # blocks per batch
    NKC = HID // P              # 16 hid chunks
    NTOKB = s_len // P          # k sub-chunks per batch
    R = b_count * s_len

    nc = bacc.Bacc(None, target_bir_lowering=False)

    hidT_d = nc.dram_tensor("hidT", [HID, R], bf16, kind="ExternalInput")
    whT_d = nc.dram_tensor("whT", [HID, OC], bf16, kind="ExternalInput")
    wkvT_d = nc.dram_tensor("wkvT", [HID, KVD], bf16, kind="ExternalInput")
    woT_d = nc.dram_tensor("woT", [OCP, HID], bf16, kind="ExternalInput")
    ropeT_d = nc.dram_tensor("ropeT", [P, s_len], fp32, kind="ExternalInput")
    maskT_d = nc.dram_tensor("maskT", [P, 4, SBLK], bf16,
                             kind="ExternalInput")
    out_d = nc.dram_tensor("out_part", [R, HID], fp32, kind="ExternalOutput")
    if debug:
        NTOKB_ = s_len // P
        dbg_v = nc.dram_tensor("dbg_v", [P, NTOKB_, 577], bf16,
                               kind="ExternalOutput")
        dbg_key = nc.dram_tensor("dbg_key", [P, 5, s_len], bf16,
                                 kind="ExternalOutput")
        dbg_q = nc.dram_tensor("dbg_q", [P, 10, SBLK], bf16,
                               kind="ExternalOutput")
        dbg_ept = nc.dram_tensor("dbg_ept", [P, SBLK], bf16,
                                 kind="ExternalOutput")
        dbg_avt = nc.dram_tensor("dbg_avt", [P, 4, 65], fp32,
                                 kind="ExternalOutput")

    with tile.TileContext(nc) as tc:
        with (
            tc.tile_pool(name="singles", bufs=1) as singles,
            tc.tile_pool(name="batch", bufs=1) as batch,
            tc.tile_pool(name="work", bufs=1) as work,
            tc.tile_pool(name="strm", bufs=1) as strm,
            tc.tile_pool(name="stats", bufs=8) as stats,
            tc.tile_pool(name="psQ", bufs=2, space="PSUM") as psQ,
            tc.tile_pool(name="psAV", bufs=4, space="PSUM") as psAV,
            tc.tile_pool(name="psT", bufs=2, space="PSUM") as psT,
        ):
            # ---- resident weights / tables ----
            # Spread the big one-time loads across DMA queues (sync/vector/
            # scalar issue to different queues) and order so the first
            # consumers (kv GEMM: wkvT+hidT) are ready earliest.
            wkvT = singles.tile([P, NKC, KVD], bf16, name="wkvT")
            nc.sync.dma_start(
                out=wkvT[:, :, :],
                in_=wkvT_d.rearrange("(a p) m -> p a m", p=P))
            whT = singles.tile([P, NKC, OC], bf16, name="whT")
            nc.scalar.dma_start(
                out=whT[:, :, :],
                in_=whT_d.rearrange("(a p) m -> p a m", p=P))
            woT = singles.tile([P, 10, HID], bf16, name="woT")
            nc.gpsimd.dma_start(
                out=woT[:, :, :],
                in_=woT_d.rearrange("(a p) m -> p a m", p=P))
            ropeT = singles.tile([P, s_len], fp32, name="ropeT")
            nc.sync.dma_start(out=ropeT[:, :], in_=ropeT_d[:, :])
            maskT = singles.tile([P, 4, SBLK], bf16, name="maskT")
            nc.sync.dma_start(out=maskT[:, :, :], in_=maskT_d[:, :, :])
            identb = singles.tile([P, P], bf16, name="identb")
            make_identity(nc, identb[:, :])

            def rope_apply(dst_hi, dst_lo, src0, src32, cols):
                """dst rows <- rope(src [64 PSUM rows; src0=rows 0:32,
                src32=rows 32:64 at any partition base]).

                Table: rows 0:64 cos, 64:128 swizzled sin (see make_in_maps).
                m2 is written half-swapped so every SBUF+SBUF op below has
                equal input base partitions (a walrus verifier requirement).
                """
                m1 = strm.tile([64, SBLK], bf16, tag="m1", bufs=2, name="m1")
                m2 = strm.tile([64, SBLK], bf16, tag="m2", bufs=2, name="m2")
                nc.vector.tensor_mul(m1[0:32, :], src0, ropeT[0:32, cols])
                nc.vector.tensor_mul(m1[32:64, :], src32, ropeT[32:64, cols])
                nc.vector.tensor_mul(m2[32:64, :], src0, ropeT[64:96, cols])
                nc.vector.tensor_mul(m2[0:32, :], src32, ropeT[96:128, cols])
                nc.vector.tensor_sub(dst_hi, m1[0:32, :], m2[0:32, :])
                nc.vector.tensor_add(dst_lo, m1[32:64, :], m2[32:64, :])

            for b in range(b_count):
                keyT = batch.tile([P, 5, s_len], bf16, tag="keyT",
                                  name="keyT")
                # rows 64:128 of slot4 are never real data, but the V
                # transpose reads the full 128 rows (K=64 PE transposes
                # fail on hw); keep them finite.
                nc.gpsimd.memset(keyT[64:128, 4, :], 0.0)
                V = batch.tile([P, NTOKB, 577], bf16, tag="V", name="V")
                nc.gpsimd.memset(V[:, :, 576:577], 1.0)

                for blk in range(NB):
                    tok0 = blk * SBLK
                    rows0 = b * s_len + tok0
                    bcols = slice(tok0, tok0 + SBLK)

                    hidT = work.tile([P, NKC, SBLK], bf16, tag="hidT",
                                     bufs=1, name="hidT")
                    nc.sync.dma_start(
                        out=hidT[:, :, :],
                        in_=hidT_d[:, rows0:rows0 + SBLK].rearrange(
                            "(a p) s -> p a s", p=P))

                    # ---- kv projection -> keyT slots (+rope) + vk0 ----
                    # vk0 holds V dims 0:128 d-major (= [v_rope; nope 0:64])
                    # so every V transpose below is a full-K=128 transpose
                    # (K=64 PE transposes fail at runtime on this backend).
                    vk0 = work.tile([P, SBLK], bf16, tag="vk0",
                                    bufs=2, name="vk0")
                    for c in range(5):
                        ps = psQ.tile([P, SBLK], fp32, tag="psQ", name="psQ")
                        for a in range(NKC):
                            nc.tensor.matmul(
                                ps[:, :], wkvT[:, a, c * P:(c + 1) * P],
                                hidT[:, a, :],
                                start=(a == 0), stop=(a == NKC - 1))
                        if c == 0:
                            rope_apply(keyT[0:32, 0, bcols],
                                       keyT[32:64, 0, bcols],
                                       ps[0:32, :], ps[32:64, :], bcols)
                            nc.vector.tensor_copy(out=vk0[0:64, :],
                                                  in_=ps[64:128, :])
                        else:
                            nc.vector.tensor_copy(
                                out=keyT[64:128, c - 1, bcols],
                                in_=ps[0:64, :])
                            nc.vector.tensor_copy(
                                out=keyT[0:64, c, bcols],
                                in_=ps[64:128, :])
                            if c == 1:
                                nc.scalar.copy(out=vk0[64:128, :],
                                               in_=ps[0:64, :])

                    # ---- fused q projection -> queryT slots (+rope) ----
                    queryT = work.tile([P, 10, SBLK], bf16, tag="queryT",
                                       bufs=1, name="queryT")
                    for c in range(9):
                        ps = psQ.tile([P, SBLK], fp32, tag="psQ", name="psQ")
                        for a in range(NKC):
                            nc.tensor.matmul(
                                ps[:, :], whT[:, a, c * P:(c + 1) * P],
                                hidT[:, a, :],
                                start=(a == 0), stop=(a == NKC - 1))
                        if c == 0:
                            rope_apply(queryT[0:32, 0, :],
                                       queryT[32:64, 0, :],
                                       ps[0:32, :], ps[32:64, :], bcols)
                            nc.scalar.copy(out=queryT[64:128, 0, :],
                                           in_=ps[64:128, :])
                        elif c < 4:
                            nc.scalar.copy(out=queryT[:, c, :], in_=ps[:, :])
                        elif c == 4:
                            nc.scalar.copy(out=queryT[0:64, 4, :],
                                           in_=ps[0:64, :])
                            rope_apply(queryT[0:32, 5, :],
                                       queryT[32:64, 5, :],
                                       ps[64:96, :], ps[96:128, :], bcols)
                        else:
                            nc.vector.tensor_copy(
                                out=queryT[64:128, c, :], in_=ps[0:64, :])
                            nc.vector.tensor_copy(
                                out=queryT[0:64, c + 1, :], in_=ps[64:128, :])

                    # ---- V assembly via PE transpose (k-major); emitted
                    # after the q GEMM so keyT evacuations complete while
                    # the PE streams q matmuls (no PE stall on DVE) ----
                    for sc in range(4):
                        tkc = blk * 4 + sc
                        kcols = slice(tok0 + sc * P, tok0 + (sc + 1) * P)
                        lcols = slice(sc * P, (sc + 1) * P)
                        tr = psT.tile([P, 640], bf16, tag="psT", bufs=1,
                                      name="trV")
                        nc.tensor.transpose(tr[:, 0:128], vk0[:, lcols],
                                            identb[:, :])
                        for c in range(1, 4):
                            nc.tensor.transpose(tr[:, c * P:(c + 1) * P],
                                                keyT[:, c, kcols],
                                                identb[:, :])
                        nc.tensor.transpose(tr[:, 512:640],
                                            keyT[:, 4, kcols],
                                            identb[:, :])
                        nc.scalar.copy(out=V[:, tkc, 0:576],
                                       in_=tr[:, 0:576])

                    if debug and b == 0 and blk == 0:
                        nc.gpsimd.memset(keyT[64:128, 4, :], 0.0)
                        nc.gpsimd.memset(queryT[64:128, 4, :], 0.0)
                        nc.gpsimd.memset(queryT[64:128, 9, :], 0.0)
                        nc.gpsimd.dma_start(out=dbg_v[:, :, :],
                                            in_=V[:, :, :])
                        nc.gpsimd.dma_start(out=dbg_key[:, :, :],
                                            in_=keyT[:, :, :])
                        nc.gpsimd.dma_start(out=dbg_q[:, :, :],
                                            in_=queryT[:, :, :])

                    # ---- attention (2 heads), scores transposed ----
                    attnoutT = work.tile([P, 10, SBLK], bf16, tag="attnoutT",
                                         bufs=1, name="attnoutT")
                    nkt = (blk + 1) * 4
                    for hh in range(HPC if stage >= 2 else 0):
                        avm = [psAV.tile([P, SBLK], fp32, tag="psAV",
                                         name="avm") for _ in range(4)]
                        avt = psT.tile([P, 4, 65], fp32, tag="avt", bufs=1,
                                       name="avt")

                        def qk(kt):
                            ps = psQ.tile([P, SBLK], fp32, tag="psQ",
                                          name="psS")
                            for s_i in range(5):
                                kw = 64 if s_i == 4 else P
                                nc.tensor.matmul(
                                    ps[:, :],
                                    keyT[0:kw, s_i, kt * P:(kt + 1) * P],
                                    queryT[0:kw, hh * 5 + s_i, :],
                                    start=(s_i == 0), stop=(s_i == 4))
                            return ps

                        # software-pipelined by one kt: the PE runs
                        # QK(kt+1) while the ACT exp of kt is in flight, so
                        # AV(kt) rarely waits and the PE stays at high
                        # p-state.
                        ps_cur = qk(0)
                        for kt in range(nkt):
                            ps_nxt = qk(kt + 1) if kt + 1 < nkt else None
                            ept = strm.tile([P, SBLK], bf16, tag="ept",
                                            bufs=3, name="ept")
                            nc.scalar.activation(ept[:, :], ps_cur[:, :],
                                                 Exp, scale=SCALE)
                            if kt // 4 == blk:
                                nc.vector.tensor_mul(
                                    ept[:, :], ept[:, :],
                                    maskT[:, kt % 4, :])
                            if debug and b == 0 and blk == 0 and hh == 0 \
                                    and kt == 0:
                                nc.gpsimd.dma_start(out=dbg_ept[:, :],
                                                    in_=ept[:, :])
                            for qs in range(4):
                                st = (kt == 0)
                                sp = (kt == nkt - 1)
                                nc.tensor.matmul(
                                    avm[qs][:, :],
                                    ept[:, qs * P:(qs + 1) * P],
                                    V[:, kt, 0:512],
                                    start=st, stop=sp, skip_group_check=True)
                                # start=True zero-flags the WHOLE psum bank
                                # (lazily applied on next write), so only the
                                # first group may set it; later qs groups
                                # overwrite via the pending flag it left.
                                nc.tensor.matmul(
                                    avt[:, qs, :],
                                    ept[:, qs * P:(qs + 1) * P],
                                    V[:, kt, 512:577],
                                    start=(st and qs == 0), stop=sp,
                                    skip_group_check=True)
                            ps_cur = ps_nxt
                        if debug and b == 0 and blk == 0 and hh == 0:
                            davt = work.tile([P, 4, 65], fp32, tag="davt",
                                             name="davt")
                            nc.vector.tensor_copy(out=davt[:, :, :],
                                                  in_=avt[:, :, :])
                            nc.gpsimd.dma_start(out=dbg_avt[:, :, :],
                                                in_=davt[:, :, :])
                        ao = work.tile([P, 4, 640], bf16, tag="ao", bufs=2,
                                       name="ao")
                        for qs in range(4):
                            rec = stats.tile([P, 1], fp32, tag="rec",
                                             name="rec")
                            nc.vector.reciprocal(rec[:, :],
                                                 avt[:, qs, 64:65])
                            nc.scalar.activation(ao[:, qs, 0:512],
                                                 avm[qs][:, :], Copy,
                                                 scale=rec[:, :])
                            nc.scalar.activation(ao[:, qs, 512:576],
                                                 avt[:, qs, 0:64], Copy,
                                                 scale=rec[:, :])
                            nc.gpsimd.memset(ao[:, qs, 576:640], 0.0)
                        for qs in range(4):
                            tr = psT.tile([P, 640], bf16, tag="psT", bufs=1,
                                          name="trA")
                            for c in range(5):
                                nc.tensor.transpose(
                                    tr[:, c * P:(c + 1) * P],
                                    ao[:, qs, c * P:(c + 1) * P],
                                    identb[:, :])
                            nc.vector.tensor_copy(
                                out=attnoutT[:, hh * 5:(hh + 1) * 5,
                                             qs * P:(qs + 1) * P],
                                in_=tr[:, :].rearrange("p (a c) -> p a c",
                                                       c=P))

                    # ---- out = attnoutT^T @ w_oT ----
                    for ct in range(HID // SBLK if stage >= 3 else 0):
                        for qs in range(4):
                            ps = psQ.tile([P, SBLK], fp32, tag="psQ",
                                          name="psO")
                            for s_i in range(10):
                                nc.tensor.matmul(
                                    ps[:, :],
                                    attnoutT[:, s_i, qs * P:(qs + 1) * P],
                                    woT[:, s_i, ct * SBLK:(ct + 1) * SBLK],
                                    start=(s_i == 0), stop=(s_i == 9))
                            osb = work.tile([P, SBLK], fp32, tag="osb",
                                            bufs=3, name="osb")
                            if (ct + qs) % 2 == 0:
                                nc.vector.tensor_copy(out=osb[:, :],
                                                      in_=ps[:, :])
                            else:
                                nc.scalar.copy(out=osb[:, :], in_=ps[:, :])
                            nc.gpsimd.dma_start(
                                out=out_d[rows0 + qs * P:
                                          rows0 + (qs + 1) * P,
                                          ct * SBLK:(ct + 1) * SBLK],
                                in_=osb[:, :])

    nc.compile()
    return nc


def make_in_maps(inputs, b_count=B, s_len=S):
    hidden = np.asarray(inputs["hidden_states"],
                        dtype=np.float32).reshape(b_count * s_len, HID)
    cos = np.asarray(inputs["cos"], dtype=np.float32)[0, :s_len]  # [s,64]
    sin = np.asarray(inputs["sin"], dtype=np.float32)[0, :s_len]
    w_qa = np.asarray(inputs["w_qa"], np.float32)
    w_qb = np.asarray(inputs["w_qb"], np.float32)
    w_kv = np.asarray(inputs["w_kv"], np.float32)
    w_o = np.asarray(inputs["w_o"], np.float32)

    hidT = np.ascontiguousarray(hidden.T).astype(BF)            # [HID, R]
    wkvT = np.ascontiguousarray(w_kv.T).astype(BF)              # [HID, 640]
    W_full = w_qb @ w_qa                                        # [H*576, HID]

    # rope table: rows 0:64 cos^T; rows 64:96 sin^T[32:64]; 96:128 sin^T[0:32]
    ropeT = np.ascontiguousarray(np.concatenate(
        [cos.T, sin.T[32:64], sin.T[0:32]], axis=0))            # [128, s]

    r = np.arange(P)[:, None]
    q = np.arange(SBLK)[None, :]
    maskT = np.stack([(r + 128 * j <= q) for j in range(4)],
                     axis=1).astype(BF)                         # [128,4,512]

    in_maps = []
    for c in range(N_CORES):
        W_h = W_full[c * OC:(c + 1) * OC]                       # [1152, HID]
        whT = np.ascontiguousarray(W_h.T).astype(BF)            # [HID, 1152]
        w_o_h = w_o[:, c * OC:(c + 1) * OC]                     # [HID, 1152]
        woT = np.zeros((OCP, HID), np.float32)
        for h2 in range(HPC):
            woT[h2 * 640:h2 * 640 + 576] = \
                w_o_h[:, h2 * 576:(h2 + 1) * 576].T
        in_maps.append({
            "hidT": hidT,
            "whT": whT,
            "wkvT": wkvT,
            "woT": woT.astype(BF),
            "ropeT": ropeT,
            "maskT": maskT,
        })
    return in_maps


_NC_CACHE = {}


def run_on_hw(inputs, trace=False):
    import os

    from concourse.bass_utils import run_bass_kernel_spmd

    if not trace:
        os.environ["BASS_NEVER_TRACE"] = "1"

    key = "full"
    if key not in _NC_CACHE:
        _NC_CACHE[key] = build_nc()
    nc = _NC_CACHE[key]
    in_maps = make_in_maps(inputs)
    res = run_bass_kernel_spmd(nc, in_maps, core_ids=list(range(N_CORES)),
                               trace=trace)
    acc = np.zeros((B * S, HID), dtype=np.float32)
    for r in res.results:
        acc += r["out_part"]
    return acc.reshape(B, S, HID), res


def kernel(**inputs):
    out, _ = run_on_hw(inputs, trace=False)
    return out


# revision 39
# speedup vs baseline: 1.0730x; 1.0730x over previous
"""MLA absorbed-QKVO attention kernel for Trainium2 (8 NeuronCores), v3.

Sharding: heads (H=16) tensor-parallel across 8 cores, 2 heads/core.
Host fuses W_h = w_qb_h @ w_qa (per-core), pre-transposes/casts all
weights + hidden to bf16, and builds swizzled rope tables + causal
masks. Each core computes a partial output (its 2 heads through w_o);
the host sums the 8 partials.

Device dataflow is weights-stationary / d-major throughout:
  q^T       = W_hT  x hidT      (PSUM -> queryT slots, rope on d-rows)
  kv^T      = w_kvT x hidT      (PSUM -> keyT slots + V via PE transpose)
  scores^T  = keyT^T x queryT   (PSUM -> exp -> P^T bf16, no-max softmax)
  attnout   = P^T^T x V         (q-major PSUM; 1/sum via per-partition
                                 scale at evacuation; ones-column of V
                                 gives the softmax denominator for free)
  out       = attnoutT^T x w_oT (after a small PE transpose of attnout)
"""

import sys

import numpy as np

if "/opt/trn_rl_repo" not in sys.path:
    sys.path.insert(0, "/opt/trn_rl_repo")

import ml_dtypes

BF = ml_dtypes.bfloat16

B, S, HID = 2, 2048, 2048
H = 16
QK_ROPE = 64
KVR = 512
QLR = 1536
KVD = 640
DHEAD = 576
N_CORES = 8
HPC = H // N_CORES
OC = HPC * DHEAD      # 1152
OCP = HPC * 640       # 1280 (per-head padded to 5x128)
SCALE = 1.0 / float(np.sqrt(128.0))

P = 128
SBLK = 512


def build_nc(b_count=B, s_len=S, debug=False, stage=3):
    import concourse.bass as bass  # noqa: F401
    import concourse.mybir as mybir
    import concourse.tile as tile
    from concourse import bacc
    from concourse.masks import make_identity

    fp32 = mybir.dt.float32
    bf16 = mybir.dt.bfloat16
    Exp = mybir.ActivationFunctionType.Exp
    Copy = mybir.ActivationFunctionType.Copy

    NB = s_len // SBLK          # blocks per batch
    NKC = HID // P              # 16 hid chunks
    NTOKB = s_len // P          # k sub-chunks per batch
    R = b_count * s_len

    nc = bacc.Bacc(None, target_bir_lowering=False)

    hidT_d = nc.dram_tensor("hidT", [HID, R], bf16, kind="ExternalInput")
    whT_d = nc.dram_tensor("whT", [HID, OC], bf16, kind="ExternalInput")
    wkvT_d = nc.dram_tensor("wkvT", [HID, KVD], bf16, kind="ExternalInput")
    woT_d = nc.dram_tensor("woT", [OCP, HID], bf16, kind="ExternalInput")
    ropeT_d = nc.dram_tensor("ropeT", [P, s_len], fp32, kind="ExternalInput")
    maskT_d = nc.dram_tensor("maskT", [P, 4, SBLK], bf16,
                             kind="ExternalInput")
    out_d = nc.dram_tensor("out_part", [R, HID], fp32, kind="ExternalOutput")
    if debug:
        NTOKB_ = s_len // P
        dbg_v = nc.dram_tensor("dbg_v", [P, NTOKB_, 577], bf16,
                               kind="ExternalOutput")
        dbg_key = nc.dram_tensor("dbg_key", [P, 5, s_len], bf16,
                                 kind="ExternalOutput")
        dbg_q = nc.dram_tensor("dbg_q", [P, 10, SBLK], bf16,
                               kind="ExternalOutput")
        dbg_ept = nc.dram_tensor("dbg_ept", [P, SBLK], bf16,
                                 kind="ExternalOutput")
        dbg_avt = nc.dram_tensor("dbg_avt", [P, 4, 65], fp32,
                                 kind="ExternalOutput")

    with tile.TileContext(nc) as tc:
        with (
            tc.tile_pool(name="singles", bufs=1) as singles,
            tc.tile_pool(name="batch", bufs=1) as batch,
            tc.tile_pool(name="work", bufs=1) as work,
            tc.tile_pool(name="strm", bufs=1) as strm,
            tc.tile_pool(name="stats", bufs=8) as stats,
            tc.tile_pool(name="psQ", bufs=2, space="PSUM") as psQ,
            tc.tile_pool(name="psAV", bufs=4, space="PSUM") as psAV,
            tc.tile_pool(name="psT", bufs=2, space="PSUM") as psT,
        ):
            # ---- resident weights / tables ----
            # Spread the big one-time loads across DMA queues (sync/vector/
            # scalar issue to different queues) and order so the first
            # consumers (kv GEMM: wkvT+hidT) are ready earliest.
            wkvT = singles.tile([P, NKC, KVD], bf16, name="wkvT")
            nc.sync.dma_start(
                out=wkvT[:, :, :],
                in_=wkvT_d.rearrange("(a p) m -> p a m", p=P))
            whT = singles.tile([P, NKC, OC], bf16, name="whT")
            nc.scalar.dma_start(
                out=whT[:, :, :],
                in_=whT_d.rearrange("(a p) m -> p a m", p=P))
            woT = singles.tile([P, 10, HID], bf16, name="woT")
            nc.scalar.dma_start(
                out=woT[:, :, :],
                in_=woT_d.rearrange("(a p) m -> p a m", p=P))
            ropeT = singles.tile([P, s_len], fp32, name="ropeT")
            nc.sync.dma_start(out=ropeT[:, :], in_=ropeT_d[:, :])
            maskT = singles.tile([P, 4, SBLK], bf16, name="maskT")
            nc.sync.dma_start(out=maskT[:, :, :], in_=maskT_d[:, :, :])
            identb = singles.tile([P, P], bf16, name="identb")
            make_identity(nc, identb[:, :])

            def rope_apply(dst_hi, dst_lo, src0, src32, cols):
                """dst rows <- rope(src [64 PSUM rows; src0=rows 0:32,
                src32=rows 32:64 at any partition base]).

                Table: rows 0:64 cos, 64:128 swizzled sin (see make_in_maps).
                m2 is written half-swapped so every SBUF+SBUF op below has
                equal input base partitions (a walrus verifier requirement).
                """
                m1 = strm.tile([64, SBLK], bf16, tag="m1", bufs=2, name="m1")
                m2 = strm.tile([64, SBLK], bf16, tag="m2", bufs=2, name="m2")
                nc.vector.tensor_mul(m1[0:32, :], src0, ropeT[0:32, cols])
                nc.vector.tensor_mul(m1[32:64, :], src32, ropeT[32:64, cols])
                nc.vector.tensor_mul(m2[32:64, :], src0, ropeT[64:96, cols])
                nc.vector.tensor_mul(m2[0:32, :], src32, ropeT[96:128, cols])
                nc.vector.tensor_sub(dst_hi, m1[0:32, :], m2[0:32, :])
                nc.vector.tensor_add(dst_lo, m1[32:64, :], m2[32:64, :])

            for b in range(b_count):
                keyT = batch.tile([P, 5, s_len], bf16, tag="keyT",
                                  name="keyT")
                # rows 64:128 of slot4 are never real data, but the V
                # transpose reads the full 128 rows (K=64 PE transposes
                # fail on hw); keep them finite.
                nc.gpsimd.memset(keyT[64:128, 4, :], 0.0)
                V = batch.tile([P, NTOKB, 577], bf16, tag="V", name="V")
                nc.gpsimd.memset(V[:, :, 576:577], 1.0)

                for blk in range(NB):
                    tok0 = blk * SBLK
                    rows0 = b * s_len + tok0
                    bcols = slice(tok0, tok0 + SBLK)

                    hidT = work.tile([P, NKC, SBLK], bf16, tag="hidT",
                                     bufs=1, name="hidT")
                    nc.sync.dma_start(
                        out=hidT[:, :, :],
                        in_=hidT_d[:, rows0:rows0 + SBLK].rearrange(
                            "(a p) s -> p a s", p=P))

                    # ---- kv projection -> keyT slots (+rope) + vk0 ----
                    # vk0 holds V dims 0:128 d-major (= [v_rope; nope 0:64])
                    # so every V transpose below is a full-K=128 transpose
                    # (K=64 PE transposes fail at runtime on this backend).
                    vk0 = work.tile([P, SBLK], bf16, tag="vk0",
                                    bufs=2, name="vk0")
                    for c in range(5):
                        ps = psQ.tile([P, SBLK], fp32, tag="psQ", name="psQ")
                        for a in range(NKC):
                            nc.tensor.matmul(
                                ps[:, :], wkvT[:, a, c * P:(c + 1) * P],
                                hidT[:, a, :],
                                start=(a == 0), stop=(a == NKC - 1))
                        if c == 0:
                            rope_apply(keyT[0:32, 0, bcols],
                                       keyT[32:64, 0, bcols],
                                       ps[0:32, :], ps[32:64, :], bcols)
                            nc.vector.tensor_copy(out=vk0[0:64, :],
                                                  in_=ps[64:128, :])
                        else:
                            nc.vector.tensor_copy(
                                out=keyT[64:128, c - 1, bcols],
                                in_=ps[0:64, :])
                            nc.vector.tensor_copy(
                                out=keyT[0:64, c, bcols],
                                in_=ps[64:128, :])
                            if c == 1:
                                nc.scalar.copy(out=vk0[64:128, :],
                                               in_=ps[0:64, :])

                    # ---- fused q projection -> queryT slots (+rope) ----
                    queryT = work.tile([P, 10, SBLK], bf16, tag="queryT",
                                       bufs=1, name="queryT")
                    for c in range(9):
                        ps = psQ.tile([P, SBLK], fp32, tag="psQ", name="psQ")
                        for a in range(NKC):
                            nc.tensor.matmul(
                                ps[:, :], whT[:, a, c * P:(c + 1) * P],
                                hidT[:, a, :],
                                start=(a == 0), stop=(a == NKC - 1))
                        if c == 0:
                            rope_apply(queryT[0:32, 0, :],
                                       queryT[32:64, 0, :],
                                       ps[0:32, :], ps[32:64, :], bcols)
                            nc.scalar.copy(out=queryT[64:128, 0, :],
                                           in_=ps[64:128, :])
                        elif c < 4:
                            nc.scalar.copy(out=queryT[:, c, :], in_=ps[:, :])
                        elif c == 4:
                            nc.scalar.copy(out=queryT[0:64, 4, :],
                                           in_=ps[0:64, :])
                            rope_apply(queryT[0:32, 5, :],
                                       queryT[32:64, 5, :],
                                       ps[64:96, :], ps[96:128, :], bcols)
                        else:
                            nc.vector.tensor_copy(
                                out=queryT[64:128, c, :], in_=ps[0:64, :])
                            nc.vector.tensor_copy(
                                out=queryT[0:64, c + 1, :], in_=ps[64:128, :])

                    # ---- V assembly via PE transpose (k-major); emitted
                    # after the q GEMM so keyT evacuations complete while
                    # the PE streams q matmuls (no PE stall on DVE) ----
                    for sc in range(4):
                        tkc = blk * 4 + sc
                        kcols = slice(tok0 + sc * P, tok0 + (sc + 1) * P)
                        lcols = slice(sc * P, (sc + 1) * P)
                        tr = psT.tile([P, 640], bf16, tag="psT", bufs=1,
                                      name="trV")
                        nc.tensor.transpose(tr[:, 0:128], vk0[:, lcols],
                                            identb[:, :])
                        for c in range(1, 4):
                            nc.tensor.transpose(tr[:, c * P:(c + 1) * P],
                                                keyT[:, c, kcols],
                                                identb[:, :])
                        nc.tensor.transpose(tr[:, 512:640],
                                            keyT[:, 4, kcols],
                                            identb[:, :])
                        nc.scalar.copy(out=V[:, tkc, 0:576],
                                       in_=tr[:, 0:576])

                    if debug and b == 0 and blk == 0:
                        nc.gpsimd.memset(keyT[64:128, 4, :], 0.0)
                        nc.gpsimd.memset(queryT[64:128, 4, :], 0.0)
                        nc.gpsimd.memset(queryT[64:128, 9, :], 0.0)
                        nc.gpsimd.dma_start(out=dbg_v[:, :, :],
                                            in_=V[:, :, :])
                        nc.gpsimd.dma_start(out=dbg_key[:, :, :],
                                            in_=keyT[:, :, :])
                        nc.gpsimd.dma_start(out=dbg_q[:, :, :],
                                            in_=queryT[:, :, :])

                    # ---- attention (2 heads), scores transposed ----
                    attnoutT = work.tile([P, 10, SBLK], bf16, tag="attnoutT",
                                         bufs=1, name="attnoutT")
                    nkt = (blk + 1) * 4
                    for hh in range(HPC if stage >= 2 else 0):
                        avm = [psAV.tile([P, SBLK], fp32, tag="psAV",
                                         name="avm") for _ in range(4)]
                        avt = psT.tile([P, 4, 65], fp32, tag="avt", bufs=1,
                                       name="avt")

                        def qk(kt):
                            ps = psQ.tile([P, SBLK], fp32, tag="psQ",
                                          name="psS")
                            for s_i in range(5):
                                kw = 64 if s_i == 4 else P
                                nc.tensor.matmul(
                                    ps[:, :],
                                    keyT[0:kw, s_i, kt * P:(kt + 1) * P],
                                    queryT[0:kw, hh * 5 + s_i, :],
                                    start=(s_i == 0), stop=(s_i == 4))
                            return ps

                        # software-pipelined by one kt: the PE runs
                        # QK(kt+1) while the ACT exp of kt is in flight, so
                        # AV(kt) rarely waits and the PE stays at high
                        # p-state.
                        ps_cur = qk(0)
                        for kt in range(nkt):
                            ps_nxt = qk(kt + 1) if kt + 1 < nkt else None
                            ept = strm.tile([P, SBLK], bf16, tag="ept",
                                            bufs=3, name="ept")
                            nc.scalar.activation(ept[:, :], ps_cur[:, :],
                                                 Exp, scale=SCALE)
                            if kt // 4 == blk:
                                nc.vector.tensor_mul(
                                    ept[:, :], ept[:, :],
                                    maskT[:, kt % 4, :])
                            if debug and b == 0 and blk == 0 and hh == 0 \
                                    and kt == 0:
                                nc.gpsimd.dma_start(out=dbg_ept[:, :],
                                                    in_=ept[:, :])
                            for qs in range(4):
                                st = (kt == 0)
                                sp = (kt == nkt - 1)
                                nc.tensor.matmul(
                                    avm[qs][:, :],
                                    ept[:, qs * P:(qs + 1) * P],
                                    V[:, kt, 0:512],
                                    start=st, stop=sp, skip_group_check=True)
                                # start=True zero-flags the WHOLE psum bank
                                # (lazily applied on next write), so only the
                                # first group may set it; later qs groups
                                # overwrite via the pending flag it left.
                                nc.tensor.matmul(
                                    avt[:, qs, :],
                                    ept[:, qs * P:(qs + 1) * P],
                                    V[:, kt, 512:577],
                                    start=(st and qs == 0), stop=sp,
                                    skip_group_check=True)
                            ps_cur = ps_nxt
                        if debug and b == 0 and blk == 0 and hh == 0:
                            davt = work.tile([P, 4, 65], fp32, tag="davt",
                                             name="davt")
                            nc.vector.tensor_copy(out=davt[:, :, :],
                                                  in_=avt[:, :, :])
                            nc.gpsimd.dma_start(out=dbg_avt[:, :, :],
                                                in_=davt[:, :, :])
                        ao = work.tile([P, 4, 640], bf16, tag="ao", bufs=2,
                                       name="ao")
                        for qs in range(4):
                            rec = stats.tile([P, 1], fp32, tag="rec",
                                             name="rec")
                            nc.vector.reciprocal(rec[:, :],
                                                 avt[:, qs, 64:65])
                            nc.scalar.activation(ao[:, qs, 0:512],
                                                 avm[qs][:, :], Copy,
                                                 scale=rec[:, :])
                            nc.scalar.activation(ao[:, qs, 512:576],
                                                 avt[:, qs, 0:64], Copy,
                                                 scale=rec[:, :])
                            nc.gpsimd.memset(ao[:, qs, 576:640], 0.0)
                        for qs in range(4):
                            tr = psT.tile([P, 640], bf16, tag="psT", bufs=1,
                                          name="trA")
                            for c in range(5):
                                nc.tensor.transpose(
                                    tr[:, c * P:(c + 1) * P],
                                    ao[:, qs, c * P:(c + 1) * P],
                                    identb[:, :])
                            nc.vector.tensor_copy(
                                out=attnoutT[:, hh * 5:(hh + 1) * 5,
                                             qs * P:(qs + 1) * P],
                                in_=tr[:, :].rearrange("p (a c) -> p a c",
                                                       c=P))

                    # ---- out = attnoutT^T @ w_oT ----
                    for ct in range(HID // SBLK if stage >= 3 else 0):
                        for qs in range(4):
                            ps = psQ.tile([P, SBLK], fp32, tag="psQ",
                                          name="psO")
                            for s_i in range(10):
                                nc.tensor.matmul(
                                    ps[:, :],
                                    attnoutT[:, s_i, qs * P:(qs + 1) * P],
                                    woT[:, s_i, ct * SBLK:(ct + 1) * SBLK],
                                    start=(s_i == 0), stop=(s_i == 9))
                            osb = work.tile([P, SBLK], fp32, tag="osb",
                                            bufs=3, name="osb")
                            if (ct + qs) % 2 == 0:
                                nc.vector.tensor_copy(out=osb[:, :],
                                                      in_=ps[:, :])
                            else:
                                nc.scalar.copy(out=osb[:, :], in_=ps[:, :])
                            nc.gpsimd.dma_start(
                                out=out_d[rows0 + qs * P:
                                          rows0 + (qs + 1) * P,
                                          ct * SBLK:(ct + 1) * SBLK],
                                in_=osb[:, :])

    nc.compile()
    return nc


def make_in_maps(inputs, b_count=B, s_len=S):
    hidden = np.asarray(inputs["hidden_states"],
                        dtype=np.float32).reshape(b_count * s_len, HID)
    cos = np.asarray(inputs["cos"], dtype=np.float32)[0, :s_len]  # [s,64]
    sin = np.asarray(inputs["sin"], dtype=np.float32)[0, :s_len]
    w_qa = np.asarray(inputs["w_qa"], np.float32)
    w_qb = np.asarray(inputs["w_qb"], np.float32)
    w_kv = np.asarray(inputs["w_kv"], np.float32)
    w_o = np.asarray(inputs["w_o"], np.float32)

    hidT = np.ascontiguousarray(hidden.T).astype(BF)            # [HID, R]
    wkvT = np.ascontiguousarray(w_kv.T).astype(BF)              # [HID, 640]
    W_full = w_qb @ w_qa                                        # [H*576, HID]

    # rope table: rows 0:64 cos^T; rows 64:96 sin^T[32:64]; 96:128 sin^T[0:32]
    ropeT = np.ascontiguousarray(np.concatenate(
        [cos.T, sin.T[32:64], sin.T[0:32]], axis=0))            # [128, s]

    r = np.arange(P)[:, None]
    q = np.arange(SBLK)[None, :]
    maskT = np.stack([(r + 128 * j <= q) for j in range(4)],
                     axis=1).astype(BF)                         # [128,4,512]

    in_maps = []
    for c in range(N_CORES):
        W_h = W_full[c * OC:(c + 1) * OC]                       # [1152, HID]
        whT = np.ascontiguousarray(W_h.T).astype(BF)            # [HID, 1152]
        w_o_h = w_o[:, c * OC:(c + 1) * OC]                     # [HID, 1152]
        woT = np.zeros((OCP, HID), np.float32)
        for h2 in range(HPC):
            woT[h2 * 640:h2 * 640 + 576] = \
                w_o_h[:, h2 * 576:(h2 + 1) * 576].T
        in_maps.append({
            "hidT": hidT,
            "whT": whT,
            "wkvT": wkvT,
            "woT": woT.astype(BF),
            "ropeT": ropeT,
            "maskT": maskT,
        })
    return in_maps


_NC_CACHE = {}


def run_on_hw(inputs, trace=False):
    import os

    from concourse.bass_utils import run_bass_kernel_spmd

    if not trace:
        os.environ["BASS_NEVER_TRACE"] = "1"

    key = "full"
    if key not in _NC_CACHE:
        _NC_CACHE[key] = build_nc()
    nc = _NC_CACHE[key]
    in_maps = make_in_maps(inputs)
    res = run_bass_kernel_spmd(nc, in_maps, core_ids=list(range(N_CORES)),
                               trace=trace)
    acc = np.zeros((B * S, HID), dtype=np.float32)
    for r in res.results:
        acc += r["out_part"]
    return acc.reshape(B, S, HID), res


def kernel(**inputs):
    out, _ = run_on_hw(inputs, trace=False)
    return out


# revision 41
# speedup vs baseline: 1.1117x; 1.0361x over previous
"""MLA absorbed-QKVO attention kernel for Trainium2 (8 NeuronCores), v3.

Sharding: heads (H=16) tensor-parallel across 8 cores, 2 heads/core.
Host fuses W_h = w_qb_h @ w_qa (per-core), pre-transposes/casts all
weights + hidden to bf16, and builds swizzled rope tables + causal
masks. Each core computes a partial output (its 2 heads through w_o);
the host sums the 8 partials.

Device dataflow is weights-stationary / d-major throughout:
  q^T       = W_hT  x hidT      (PSUM -> queryT slots, rope on d-rows)
  kv^T      = w_kvT x hidT      (PSUM -> keyT slots + V via PE transpose)
  scores^T  = keyT^T x queryT   (PSUM -> exp -> P^T bf16, no-max softmax)
  attnout   = P^T^T x V         (q-major PSUM; 1/sum via per-partition
                                 scale at evacuation; ones-column of V
                                 gives the softmax denominator for free)
  out       = attnoutT^T x w_oT (after a small PE transpose of attnout)
"""

import sys

import numpy as np

if "/opt/trn_rl_repo" not in sys.path:
    sys.path.insert(0, "/opt/trn_rl_repo")

import ml_dtypes

BF = ml_dtypes.bfloat16

B, S, HID = 2, 2048, 2048
H = 16
QK_ROPE = 64
KVR = 512
QLR = 1536
KVD = 640
DHEAD = 576
N_CORES = 8
HPC = H // N_CORES
OC = HPC * DHEAD      # 1152
OCP = HPC * 640       # 1280 (per-head padded to 5x128)
SCALE = 1.0 / float(np.sqrt(128.0))

P = 128
SBLK = 512


def build_nc(b_count=B, s_len=S, debug=False, stage=3):
    import concourse.bass as bass  # noqa: F401
    import concourse.mybir as mybir
    import concourse.tile as tile
    from concourse import bacc
    from concourse.masks import make_identity

    fp32 = mybir.dt.float32
    bf16 = mybir.dt.bfloat16
    Exp = mybir.ActivationFunctionType.Exp
    Copy = mybir.ActivationFunctionType.Copy

    NB = s_len // SBLK          # blocks per batch
    NKC = HID // P              # 16 hid chunks
    NTOKB = s_len // P          # k sub-chunks per batch
    R = b_count * s_len

    nc = bacc.Bacc(None, target_bir_lowering=False)

    hidT_d = nc.dram_tensor("hidT", [HID, R], bf16, kind="ExternalInput")
    whT_d = nc.dram_tensor("whT", [HID, OC], bf16, kind="ExternalInput")
    wkvT_d = nc.dram_tensor("wkvT", [HID, KVD], bf16, kind="ExternalInput")
    woT_d = nc.dram_tensor("woT", [OCP, HID], bf16, kind="ExternalInput")
    ropeT_d = nc.dram_tensor("ropeT", [P, s_len], fp32, kind="ExternalInput")
    maskT_d = nc.dram_tensor("maskT", [P, 4, SBLK], bf16,
                             kind="ExternalInput")
    out_d = nc.dram_tensor("out_part", [R, HID], bf16, kind="ExternalOutput")
    if debug:
        NTOKB_ = s_len // P
        dbg_v = nc.dram_tensor("dbg_v", [P, NTOKB_, 577], bf16,
                               kind="ExternalOutput")
        dbg_key = nc.dram_tensor("dbg_key", [P, 5, s_len], bf16,
                                 kind="ExternalOutput")
        dbg_q = nc.dram_tensor("dbg_q", [P, 10, SBLK], bf16,
                               kind="ExternalOutput")
        dbg_ept = nc.dram_tensor("dbg_ept", [P, SBLK], bf16,
                                 kind="ExternalOutput")
        dbg_avt = nc.dram_tensor("dbg_avt", [P, 4, 65], fp32,
                                 kind="ExternalOutput")

    with tile.TileContext(nc) as tc:
        with (
            tc.tile_pool(name="singles", bufs=1) as singles,
            tc.tile_pool(name="batch", bufs=1) as batch,
            tc.tile_pool(name="work", bufs=1) as work,
            tc.tile_pool(name="strm", bufs=1) as strm,
            tc.tile_pool(name="stats", bufs=8) as stats,
            tc.tile_pool(name="psQ", bufs=2, space="PSUM") as psQ,
            tc.tile_pool(name="psAV", bufs=4, space="PSUM") as psAV,
            tc.tile_pool(name="psT", bufs=2, space="PSUM") as psT,
        ):
            # ---- resident weights / tables ----
            # Spread the big one-time loads across DMA queues (sync/vector/
            # scalar issue to different queues) and order so the first
            # consumers (kv GEMM: wkvT+hidT) are ready earliest.
            wkvT = singles.tile([P, NKC, KVD], bf16, name="wkvT")
            nc.sync.dma_start(
                out=wkvT[:, :, :],
                in_=wkvT_d.rearrange("(a p) m -> p a m", p=P))
            whT = singles.tile([P, NKC, OC], bf16, name="whT")
            nc.scalar.dma_start(
                out=whT[:, :, :],
                in_=whT_d.rearrange("(a p) m -> p a m", p=P))
            woT = singles.tile([P, 10, HID], bf16, name="woT")
            nc.scalar.dma_start(
                out=woT[:, :, :],
                in_=woT_d.rearrange("(a p) m -> p a m", p=P))
            ropeT = singles.tile([P, s_len], fp32, name="ropeT")
            nc.sync.dma_start(out=ropeT[:, :], in_=ropeT_d[:, :])
            maskT = singles.tile([P, 4, SBLK], bf16, name="maskT")
            nc.sync.dma_start(out=maskT[:, :, :], in_=maskT_d[:, :, :])
            identb = singles.tile([P, P], bf16, name="identb")
            make_identity(nc, identb[:, :])

            def rope_apply(dst_hi, dst_lo, src0, src32, cols):
                """dst rows <- rope(src [64 PSUM rows; src0=rows 0:32,
                src32=rows 32:64 at any partition base]).

                Table: rows 0:64 cos, 64:128 swizzled sin (see make_in_maps).
                m2 is written half-swapped so every SBUF+SBUF op below has
                equal input base partitions (a walrus verifier requirement).
                """
                m1 = strm.tile([64, SBLK], bf16, tag="m1", bufs=2, name="m1")
                m2 = strm.tile([64, SBLK], bf16, tag="m2", bufs=2, name="m2")
                nc.vector.tensor_mul(m1[0:32, :], src0, ropeT[0:32, cols])
                nc.vector.tensor_mul(m1[32:64, :], src32, ropeT[32:64, cols])
                nc.vector.tensor_mul(m2[32:64, :], src0, ropeT[64:96, cols])
                nc.vector.tensor_mul(m2[0:32, :], src32, ropeT[96:128, cols])
                nc.vector.tensor_sub(dst_hi, m1[0:32, :], m2[0:32, :])
                nc.vector.tensor_add(dst_lo, m1[32:64, :], m2[32:64, :])

            for b in range(b_count):
                keyT = batch.tile([P, 5, s_len], bf16, tag="keyT",
                                  name="keyT")
                # rows 64:128 of slot4 are never real data, but the V
                # transpose reads the full 128 rows (K=64 PE transposes
                # fail on hw); keep them finite.
                nc.gpsimd.memset(keyT[64:128, 4, :], 0.0)
                V = batch.tile([P, NTOKB, 577], bf16, tag="V", name="V")
                nc.gpsimd.memset(V[:, :, 576:577], 1.0)

                for blk in range(NB):
                    tok0 = blk * SBLK
                    rows0 = b * s_len + tok0
                    bcols = slice(tok0, tok0 + SBLK)

                    hidT = work.tile([P, NKC, SBLK], bf16, tag="hidT",
                                     bufs=1, name="hidT")
                    nc.sync.dma_start(
                        out=hidT[:, :, :],
                        in_=hidT_d[:, rows0:rows0 + SBLK].rearrange(
                            "(a p) s -> p a s", p=P))

                    # ---- kv projection -> keyT slots (+rope) + vk0 ----
                    # vk0 holds V dims 0:128 d-major (= [v_rope; nope 0:64])
                    # so every V transpose below is a full-K=128 transpose
                    # (K=64 PE transposes fail at runtime on this backend).
                    vk0 = work.tile([P, SBLK], bf16, tag="vk0",
                                    bufs=2, name="vk0")
                    for c in range(5):
                        ps = psQ.tile([P, SBLK], fp32, tag="psQ", name="psQ")
                        for a in range(NKC):
                            nc.tensor.matmul(
                                ps[:, :], wkvT[:, a, c * P:(c + 1) * P],
                                hidT[:, a, :],
                                start=(a == 0), stop=(a == NKC - 1))
                        if c == 0:
                            rope_apply(keyT[0:32, 0, bcols],
                                       keyT[32:64, 0, bcols],
                                       ps[0:32, :], ps[32:64, :], bcols)
                            nc.vector.tensor_copy(out=vk0[0:64, :],
                                                  in_=ps[64:128, :])
                        else:
                            nc.vector.tensor_copy(
                                out=keyT[64:128, c - 1, bcols],
                                in_=ps[0:64, :])
                            nc.vector.tensor_copy(
                                out=keyT[0:64, c, bcols],
                                in_=ps[64:128, :])
                            if c == 1:
                                nc.scalar.copy(out=vk0[64:128, :],
                                               in_=ps[0:64, :])

                    # ---- fused q projection -> queryT slots (+rope) ----
                    queryT = work.tile([P, 10, SBLK], bf16, tag="queryT",
                                       bufs=1, name="queryT")
                    for c in range(9):
                        ps = psQ.tile([P, SBLK], fp32, tag="psQ", name="psQ")
                        for a in range(NKC):
                            nc.tensor.matmul(
                                ps[:, :], whT[:, a, c * P:(c + 1) * P],
                                hidT[:, a, :],
                                start=(a == 0), stop=(a == NKC - 1))
                        if c == 0:
                            rope_apply(queryT[0:32, 0, :],
                                       queryT[32:64, 0, :],
                                       ps[0:32, :], ps[32:64, :], bcols)
                            nc.scalar.copy(out=queryT[64:128, 0, :],
                                           in_=ps[64:128, :])
                        elif c < 4:
                            nc.scalar.copy(out=queryT[:, c, :], in_=ps[:, :])
                        elif c == 4:
                            nc.scalar.copy(out=queryT[0:64, 4, :],
                                           in_=ps[0:64, :])
                            rope_apply(queryT[0:32, 5, :],
                                       queryT[32:64, 5, :],
                                       ps[64:96, :], ps[96:128, :], bcols)
                        else:
                            nc.vector.tensor_copy(
                                out=queryT[64:128, c, :], in_=ps[0:64, :])
                            nc.vector.tensor_copy(
                                out=queryT[0:64, c + 1, :], in_=ps[64:128, :])

                    # ---- V assembly via PE transpose (k-major); emitted
                    # after the q GEMM so keyT evacuations complete while
                    # the PE streams q matmuls (no PE stall on DVE) ----
                    for sc in range(4):
                        tkc = blk * 4 + sc
                        kcols = slice(tok0 + sc * P, tok0 + (sc + 1) * P)
                        lcols = slice(sc * P, (sc + 1) * P)
                        tr = psT.tile([P, 640], bf16, tag="psT", bufs=1,
                                      name="trV")
                        nc.tensor.transpose(tr[:, 0:128], vk0[:, lcols],
                                            identb[:, :])
                        for c in range(1, 4):
                            nc.tensor.transpose(tr[:, c * P:(c + 1) * P],
                                                keyT[:, c, kcols],
                                                identb[:, :])
                        nc.tensor.transpose(tr[:, 512:640],
                                            keyT[:, 4, kcols],
                                            identb[:, :])
                        nc.scalar.copy(out=V[:, tkc, 0:576],
                                       in_=tr[:, 0:576])

                    if debug and b == 0 and blk == 0:
                        nc.gpsimd.memset(keyT[64:128, 4, :], 0.0)
                        nc.gpsimd.memset(queryT[64:128, 4, :], 0.0)
                        nc.gpsimd.memset(queryT[64:128, 9, :], 0.0)
                        nc.gpsimd.dma_start(out=dbg_v[:, :, :],
                                            in_=V[:, :, :])
                        nc.gpsimd.dma_start(out=dbg_key[:, :, :],
                                            in_=keyT[:, :, :])
                        nc.gpsimd.dma_start(out=dbg_q[:, :, :],
                                            in_=queryT[:, :, :])

                    # ---- attention (2 heads), scores transposed ----
                    attnoutT = work.tile([P, 10, SBLK], bf16, tag="attnoutT",
                                         bufs=1, name="attnoutT")
                    nkt = (blk + 1) * 4
                    for hh in range(HPC if stage >= 2 else 0):
                        avm = [psAV.tile([P, SBLK], fp32, tag="psAV",
                                         name="avm") for _ in range(4)]
                        avt = psT.tile([P, 4, 65], fp32, tag="avt", bufs=1,
                                       name="avt")

                        def qk(kt):
                            ps = psQ.tile([P, SBLK], fp32, tag="psQ",
                                          name="psS")
                            for s_i in range(5):
                                kw = 64 if s_i == 4 else P
                                nc.tensor.matmul(
                                    ps[:, :],
                                    keyT[0:kw, s_i, kt * P:(kt + 1) * P],
                                    queryT[0:kw, hh * 5 + s_i, :],
                                    start=(s_i == 0), stop=(s_i == 4))
                            return ps

                        # software-pipelined by one kt: the PE runs
                        # QK(kt+1) while the ACT exp of kt is in flight, so
                        # AV(kt) rarely waits and the PE stays at high
                        # p-state.
                        ps_cur = qk(0)
                        for kt in range(nkt):
                            ps_nxt = qk(kt + 1) if kt + 1 < nkt else None
                            ept = strm.tile([P, SBLK], bf16, tag="ept",
                                            bufs=3, name="ept")
                            nc.scalar.activation(ept[:, :], ps_cur[:, :],
                                                 Exp, scale=SCALE)
                            if kt // 4 == blk:
                                nc.vector.tensor_mul(
                                    ept[:, :], ept[:, :],
                                    maskT[:, kt % 4, :])
                            if debug and b == 0 and blk == 0 and hh == 0 \
                                    and kt == 0:
                                nc.gpsimd.dma_start(out=dbg_ept[:, :],
                                                    in_=ept[:, :])
                            for qs in range(4):
                                st = (kt == 0)
                                sp = (kt == nkt - 1)
                                nc.tensor.matmul(
                                    avm[qs][:, :],
                                    ept[:, qs * P:(qs + 1) * P],
                                    V[:, kt, 0:512],
                                    start=st, stop=sp, skip_group_check=True)
                                # start=True zero-flags the WHOLE psum bank
                                # (lazily applied on next write), so only the
                                # first group may set it; later qs groups
                                # overwrite via the pending flag it left.
                                nc.tensor.matmul(
                                    avt[:, qs, :],
                                    ept[:, qs * P:(qs + 1) * P],
                                    V[:, kt, 512:577],
                                    start=(st and qs == 0), stop=sp,
                                    skip_group_check=True)
                            ps_cur = ps_nxt
                        if debug and b == 0 and blk == 0 and hh == 0:
                            davt = work.tile([P, 4, 65], fp32, tag="davt",
                                             name="davt")
                            nc.vector.tensor_copy(out=davt[:, :, :],
                                                  in_=avt[:, :, :])
                            nc.gpsimd.dma_start(out=dbg_avt[:, :, :],
                                                in_=davt[:, :, :])
                        ao = work.tile([P, 4, 640], bf16, tag="ao", bufs=2,
                                       name="ao")
                        for qs in range(4):
                            rec = stats.tile([P, 1], fp32, tag="rec",
                                             name="rec")
                            nc.vector.reciprocal(rec[:, :],
                                                 avt[:, qs, 64:65])
                            nc.scalar.activation(ao[:, qs, 0:512],
                                                 avm[qs][:, :], Copy,
                                                 scale=rec[:, :])
                            nc.scalar.activation(ao[:, qs, 512:576],
                                                 avt[:, qs, 0:64], Copy,
                                                 scale=rec[:, :])
                            nc.gpsimd.memset(ao[:, qs, 576:640], 0.0)
                        for qs in range(4):
                            tr = psT.tile([P, 640], bf16, tag="psT", bufs=1,
                                          name="trA")
                            for c in range(5):
                                nc.tensor.transpose(
                                    tr[:, c * P:(c + 1) * P],
                                    ao[:, qs, c * P:(c + 1) * P],
                                    identb[:, :])
                            nc.vector.tensor_copy(
                                out=attnoutT[:, hh * 5:(hh + 1) * 5,
                                             qs * P:(qs + 1) * P],
                                in_=tr[:, :].rearrange("p (a c) -> p a c",
                                                       c=P))

                    # ---- out = attnoutT^T @ w_oT ----
                    for ct in range(HID // SBLK if stage >= 3 else 0):
                        for qs in range(4):
                            ps = psQ.tile([P, SBLK], fp32, tag="psQ",
                                          name="psO")
                            for s_i in range(10):
                                nc.tensor.matmul(
                                    ps[:, :],
                                    attnoutT[:, s_i, qs * P:(qs + 1) * P],
                                    woT[:, s_i, ct * SBLK:(ct + 1) * SBLK],
                                    start=(s_i == 0), stop=(s_i == 9))
                            osb = work.tile([P, SBLK], bf16, tag="osb",
                                            bufs=3, name="osb")
                            if (ct + qs) % 2 == 0:
                                nc.vector.tensor_copy(out=osb[:, :],
                                                      in_=ps[:, :])
                            else:
                                nc.scalar.copy(out=osb[:, :], in_=ps[:, :])
                            nc.gpsimd.dma_start(
                                out=out_d[rows0 + qs * P:
                                          rows0 + (qs + 1) * P,
                                          ct * SBLK:(ct + 1) * SBLK],
                                in_=osb[:, :])

    nc.compile()
    return nc


def make_in_maps(inputs, b_count=B, s_len=S):
    hidden = np.asarray(inputs["hidden_states"],
                        dtype=np.float32).reshape(b_count * s_len, HID)
    cos = np.asarray(inputs["cos"], dtype=np.float32)[0, :s_len]  # [s,64]
    sin = np.asarray(inputs["sin"], dtype=np.float32)[0, :s_len]
    w_qa = np.asarray(inputs["w_qa"], np.float32)
    w_qb = np.asarray(inputs["w_qb"], np.float32)
    w_kv = np.asarray(inputs["w_kv"], np.float32)
    w_o = np.asarray(inputs["w_o"], np.float32)

    hidT = np.ascontiguousarray(hidden.T).astype(BF)            # [HID, R]
    wkvT = np.ascontiguousarray(w_kv.T).astype(BF)              # [HID, 640]
    W_full = w_qb @ w_qa                                        # [H*576, HID]

    # rope table: rows 0:64 cos^T; rows 64:96 sin^T[32:64]; 96:128 sin^T[0:32]
    ropeT = np.ascontiguousarray(np.concatenate(
        [cos.T, sin.T[32:64], sin.T[0:32]], axis=0))            # [128, s]

    r = np.arange(P)[:, None]
    q = np.arange(SBLK)[None, :]
    maskT = np.stack([(r + 128 * j <= q) for j in range(4)],
                     axis=1).astype(BF)                         # [128,4,512]

    in_maps = []
    for c in range(N_CORES):
        W_h = W_full[c * OC:(c + 1) * OC]                       # [1152, HID]
        whT = np.ascontiguousarray(W_h.T).astype(BF)            # [HID, 1152]
        w_o_h = w_o[:, c * OC:(c + 1) * OC]                     # [HID, 1152]
        woT = np.zeros((OCP, HID), np.float32)
        for h2 in range(HPC):
            woT[h2 * 640:h2 * 640 + 576] = \
                w_o_h[:, h2 * 576:(h2 + 1) * 576].T
        in_maps.append({
            "hidT": hidT,
            "whT": whT,
            "wkvT": wkvT,
            "woT": woT.astype(BF),
            "ropeT": ropeT,
            "maskT": maskT,
        })
    return in_maps


_NC_CACHE = {}


def run_on_hw(inputs, trace=False):
    import os

    from concourse.bass_utils import run_bass_kernel_spmd

    if not trace:
        os.environ["BASS_NEVER_TRACE"] = "1"

    key = "full"
    if key not in _NC_CACHE:
        _NC_CACHE[key] = build_nc()
    nc = _NC_CACHE[key]
    in_maps = make_in_maps(inputs)
    res = run_bass_kernel_spmd(nc, in_maps, core_ids=list(range(N_CORES)),
                               trace=trace)
    acc = np.zeros((B * S, HID), dtype=np.float32)
    for r in res.results:
        acc += r["out_part"]
    return acc.reshape(B, S, HID), res


def kernel(**inputs):
    out, _ = run_on_hw(inputs, trace=False)
    return out
